# revision 38
# baseline (speedup 1.0000x reference)
"""Self-contained Trainium2 Bass kernel for GQA int8-KV-cache decode attention.

Full inputs -> shard over 8 cores (1 kv head + 4 q heads per core) ->
Bass/Tile kernel (QKV proj, RoPE, dequant, attention, out proj) ->
ReduceScatter over cores -> host concat.
"""
import math
from contextlib import ExitStack

import numpy as np
import ml_dtypes

import concourse.bass as bass
import concourse.tile as tile
from concourse import bacc, mybir, masks
from concourse.bass_utils import run_bass_kernel_spmd

bf16 = ml_dtypes.bfloat16
F32, BF16, I8 = mybir.dt.float32, mybir.dt.bfloat16, mybir.dt.int8

# Problem dims (hardcoded per spec)
B, H, NH, NKV, HD, G, T0 = 32, 4096, 32, 8, 128, 8, 4096
THETA = 10000.0
NCORE = 8
R = NH // NCORE            # q heads per core = 4
HL = (R + 2) * HD          # local qkv out cols = 768
NCH = T0 // 128            # past-token chunks = 32
PCOL = (NCH + 1) * R       # score cols = 132 (32 past chunks + 1 new) * 4
SUPER = 1024               # t superchunk size
NSUP = T0 // SUPER         # 4
INV_SQRT_HD = 1.0 / math.sqrt(HD)
# Of every 8 batches, route this many V-dequants to GPSIMD (rest on DVE)
GPSIMD_V_NB8 = 7


def set_dims(t0, super_):
    """Override token dims (for scaled-down simulation tests)."""
    global T0, SUPER, NCH, PCOL, NSUP
    T0, SUPER = t0, super_
    NCH = T0 // 128
    PCOL = (NCH + 1) * R
    NSUP = T0 // SUPER


def _emit(ctx: ExitStack, tc: tile.TileContext, io: dict):
    nc = tc.nc
    xT, wqkv, wo = io["xT"], io["wqkv"], io["wo"]
    k8T, skT, v8, sv, cs = io["k8T"], io["skT"], io["v8"], io["sv"], io["cs"]
    out_ext = io["out"]

    nsup = T0 // SUPER
    nch_sup = SUPER // 128          # chunks per superchunk = 8

    # ---------------- pools
    cpool = ctx.enter_context(tc.tile_pool(name="const", bufs=1))
    apool = ctx.enter_context(tc.tile_pool(name="phaseA", bufs=1))
    xw = ctx.enter_context(tc.tile_pool(name="xw", bufs=2))
    kp = ctx.enter_context(tc.tile_pool(name="kp", bufs=2))
    kgp = ctx.enter_context(tc.tile_pool(name="kgp", bufs=2))
    vp = ctx.enter_context(tc.tile_pool(name="vp", bufs=2))
    pp = ctx.enter_context(tc.tile_pool(name="pp", bufs=3))
    wop = ctx.enter_context(tc.tile_pool(name="wop", bufs=2))
    dram = ctx.enter_context(tc.tile_pool(name="dram", bufs=1, space="DRAM"))

    ps_io = ctx.enter_context(tc.tile_pool(name="ps_io", bufs=1, space="PSUM"))
    ps_skf = ctx.enter_context(tc.tile_pool(name="ps_skf", bufs=2, space="PSUM"))
    ps_sc = ctx.enter_context(tc.tile_pool(name="ps_sc", bufs=2, space="PSUM"))
    ps_at = ctx.enter_context(tc.tile_pool(name="ps_at", bufs=2, space="PSUM"))

    # ---------------- constants
    iden = cpool.tile([128, 128], F32)
    masks.make_identity(nc, iden[:, :])
    ones = cpool.tile([128, 1], BF16)
    nc.vector.memset(ones[:, :], 1.0)
    cosb = cpool.tile([B, 64], F32)
    sinb = cpool.tile([B, 64], F32)
    nc.sync.dma_start(cosb[:, :], cs[0:1, :].unsqueeze(1).broadcast_to([1, B, 64]))
    nc.sync.dma_start(sinb[:, :], cs[1:2, :].unsqueeze(1).broadcast_to([1, B, 64]))

    eexp = cpool.tile([16, 128], BF16)         # E[g,d]=1 iff d//8==g
    nc.sync.dma_start(eexp[:, :], io["eexp"][:, :])
    qT = cpool.tile([128, B * R], BF16)        # cols b*4+r
    kTn = cpool.tile([128, B], BF16)           # new-token K^T
    vnew = cpool.tile([B, 128], BF16)          # new-token V rows
    attn_n = cpool.tile([128, B * R], BF16)    # normalized attn, cols r*32+b
    wo_all = cpool.tile([128, R * H], BF16)    # preloaded wo rows
    vd_last = cpool.tile([128, 128], BF16)     # per-b last V chunk (row 0 only)
    nc.vector.memset(vd_last[:, :], 0.0)

    # ---------------- prefetch first K-scale tiles before the weight stream
    skc_pre = []
    for b0 in range(2):
        skc = kp.tile([16, T0], BF16, tag="sk")
        nc.scalar.dma_start(skc[:, :], skT[b0, :, :])
        skc_pre.append(skc)

    # ---------------- early K-path for b0/b1 (independent of QKV chain)
    KG = 8
    k8g = kgp.tile([128, KG * T0], I8, tag="k8")
    nc.sync.dma_start(k8g[:, :], k8T[0, :, :])
    kd_pre = []
    for b0 in range(2):
        kd = kp.tile([128, T0], BF16, tag="kd")
        k8c = k8g[:, b0 * T0:(b0 + 1) * T0]
        for chk in range(T0 // 512):
            skf_ps = ps_skf.tile([128, 512], F32, tag="skf")
            nc.tensor.matmul(skf_ps[:, :], eexp[:, :],
                             skc_pre[b0][:, chk * 512:(chk + 1) * 512],
                             start=True, stop=True)
            nc.vector.tensor_mul(kd[:, chk * 512:(chk + 1) * 512],
                                 k8c[:, chk * 512:(chk + 1) * 512],
                                 skf_ps[:, :])
        kd_pre.append(kd)

    # ---------------- phase A: QKV projection
    ps_qkv = ps_io.tile([B, HL], F32, tag="io")
    nhch = H // 128
    xc_all = apool.tile([128, nhch * B], BF16)   # col block h: x chunk h
    nc.scalar.dma_start(xc_all[:, :], xT[:, :])
    WGRP = 8                                     # h-chunks per w DMA
    for hg in range(nhch // WGRP):
        wc = xw.tile([128, WGRP * HL], BF16, tag="w")
        nc.scalar.dma_start(wc[:, :],
                             wqkv[:, hg * WGRP * HL:(hg + 1) * WGRP * HL])
        for hh in range(WGRP):
            h = hg * WGRP + hh
            xcv = xc_all[:, h * B:(h + 1) * B]
            wcv = wc[:, hh * HL:(hh + 1) * HL]
            nc.tensor.matmul(ps_qkv[:, 0:512], xcv, wcv[:, 0:512],
                             start=(h == 0), stop=(h == nhch - 1))
            nc.tensor.matmul(ps_qkv[:, 512:768], xcv, wcv[:, 512:768],
                             start=(h == 0), stop=(h == nhch - 1))

    qkv_sb = apool.tile([B, HL], F32)
    nc.vector.tensor_copy(qkv_sb[:, :], ps_qkv[:, :])

    # ---------------- phase A: RoPE on q (4 heads) + k (1 head)
    rope = apool.tile([B, 5 * 128], F32)
    t1 = qkv_sb[:, 0:640].rearrange("b (h c) -> b h c", h=5)[:, :, 0:64]
    t2 = qkv_sb[:, 0:640].rearrange("b (h c) -> b h c", h=5)[:, :, 64:128]
    o1 = rope[:, :].rearrange("b (h c) -> b h c", h=5)[:, :, 0:64]
    o2 = rope[:, :].rearrange("b (h c) -> b h c", h=5)[:, :, 64:128]
    cos3 = cosb[:, :].unsqueeze(1).broadcast_to([B, 5, 64])
    sin3 = sinb[:, :].unsqueeze(1).broadcast_to([B, 5, 64])
    m1 = apool.tile([B, 5 * 64], F32)
    m2 = apool.tile([B, 5 * 64], F32)
    m1v = m1[:, :].rearrange("b (h c) -> b h c", h=5)
    m2v = m2[:, :].rearrange("b (h c) -> b h c", h=5)
    nc.vector.tensor_mul(m1v, t1, cos3)
    nc.vector.tensor_mul(m2v, t2, sin3)
    nc.vector.tensor_sub(o1, m1v, m2v)
    nc.vector.tensor_mul(m1v, t2, cos3)
    nc.vector.tensor_mul(m2v, t1, sin3)
    nc.vector.tensor_add(o2, m1v, m2v)

    # ---------------- phase A: transposes (q heads + new k), v_new cast
    for r in range(R):
        ps_t = ps_io.tile([128, B], F32, tag="io")
        nc.tensor.transpose(ps_t[:, :], rope[:, r * 128:(r + 1) * 128],
                            iden[0:B, 0:B])
        qT_view = qT[:, :].rearrange("d (b r) -> d b r", r=R)[:, :, r]
        nc.vector.tensor_copy(qT_view, ps_t[:, :])
    ps_t = ps_io.tile([128, B], F32, tag="io")
    nc.tensor.transpose(ps_t[:, :], rope[:, 512:640], iden[0:B, 0:B])
    nc.vector.tensor_copy(kTn[:, :], ps_t[:, :])
    nc.vector.tensor_copy(vnew[:, :], qkv_sb[:, 640:768])

    # ---------------- phase B: per-batch attention
    for b in range(B):
        # --- K path: dequant + scores
        if b % KG == 0 and b > 0:
            k8g = kgp.tile([128, KG * T0], I8, tag="k8")
            nc.sync.dma_start(k8g[:, :], k8T[b // KG, :, :])
        if b == 2:
            for r in range(R):
                nc.sync.dma_start(wo_all[:, r * H:(r + 1) * H],
                                  wo[r * 128:(r + 1) * 128, :])
        ps_s = ps_sc.tile([128, 2 * PCOL], F32, tag="sc")
        if b < 2:
            kd = kd_pre[b]
        else:
            k8c = k8g[:, (b % KG) * T0:(b % KG + 1) * T0]
            skc = kp.tile([16, T0], BF16, tag="sk")
            nc.scalar.dma_start(skc[:, :], skT[b, :, :])
            kd = kp.tile([128, T0], BF16, tag="kd")
            for chk in range(T0 // 512):
                skf_ps = ps_skf.tile([128, 512], F32, tag="skf")
                nc.tensor.matmul(skf_ps[:, :], eexp[:, :],
                                 skc[:, chk * 512:(chk + 1) * 512],
                                 start=True, stop=True)
                nc.vector.tensor_mul(kd[:, chk * 512:(chk + 1) * 512],
                                     k8c[:, chk * 512:(chk + 1) * 512],
                                     skf_ps[:, :])
        for ch in range(NCH):
            nc.tensor.matmul(ps_s[:, ch * R:(ch + 1) * R],
                             kd[:, ch * 128:(ch + 1) * 128],
                             qT[:, b * R:(b + 1) * R],
                             start=True, stop=True)
        # new-token score: row 0 of last col-block; rest = -1e30 -> exp 0
        nc.vector.memset(ps_s[:, NCH * R:PCOL], -1e30)
        nc.tensor.matmul(ps_s[0:1, NCH * R:PCOL], kTn[:, b:b + 1],
                         qT[:, b * R:(b + 1) * R], start=True, stop=True)

        # --- softmax (unnormalized): p = exp(scores/sqrt(HD))
        p_b = pp.tile([128, PCOL], BF16, tag="p")
        nc.scalar.activation(p_b[:, :], ps_s[:, 0:PCOL],
                             mybir.ActivationFunctionType.Exp,
                             scale=INV_SQRT_HD)
        # column sums via ones-matmul, then fold chunks, reciprocal
        ps_m = ps_s[0:1, PCOL:2 * PCOL]
        nc.tensor.matmul(ps_m, ones[:, :], p_b[:, :], start=True, stop=True)
        red = pp.tile([1, R], F32, tag="red")
        nc.vector.tensor_reduce(red[0:1, :],
                                ps_m.rearrange("p (c r) -> p r c", r=R),
                                axis=mybir.AxisListType.X, op=mybir.AluOpType.add)
        rec4 = pp.tile([1, R], F32, tag="rec")
        nc.vector.reciprocal(rec4[0:1, :], red[0:1, :])
        rec4b = pp.tile([128, R], F32, tag="recb")
        nc.sync.dma_start(rec4b[:, :],
                          rec4[0:1, :].unsqueeze(1).broadcast_to([1, 128, R]))

        # --- V path: dequant + attention matmul (DMAs issued from ACT queue)
        ps_a = ps_at.tile([128, R], F32, tag="at")
        v8c = vp.tile([128, T0], I8, tag="v8")
        nc.scalar.dma_start(v8c[:, :], v8[b, :, :])
        svc = vp.tile([128, NCH * 16], BF16, tag="sv")
        nc.scalar.dma_start(svc[:, :], sv[b, :, :])
        vd = vp.tile([128, T0], BF16, tag="vd")
        eng = nc.gpsimd if (b % 8) < GPSIMD_V_NB8 else nc.vector
        eng.tensor_mul(
            vd[:, :].rearrange("p (s e) -> p s e", e=G),
            v8c[:, :].rearrange("p (s e) -> p s e", e=G),
            svc[:, :].unsqueeze(2).broadcast_to([128, NCH * 16, G]))
        for ch in range(NCH):
            nc.tensor.matmul(ps_a[:, :], vd[:, ch * 128:(ch + 1) * 128],
                             p_b[:, ch * R:(ch + 1) * R],
                             start=(ch == 0), stop=False)
        # new-token V contribution
        nc.scalar.dma_start(vd_last[0:1, :], vnew[b:b + 1, :])
        nc.tensor.matmul(ps_a[:, :], vd_last[:, :], p_b[:, NCH * R:PCOL],
                         start=False, stop=True)
        at_view = attn_n[:, :].rearrange("d (r b) -> d r b", b=B)[:, :, b]
        nc.vector.tensor_mul(at_view, ps_a[:, :], rec4b[:, :])

    # ---------------- phase C: output projection + collective
    partial_d = dram.tile([B, H], F32)
    rs_out = dram.tile([B // NCORE, H], F32)
    for n in range(H // 512):
        ps_o = ps_io.tile([B, 512], F32, tag="io")
        for r in range(R):
            nc.tensor.matmul(ps_o[:, :], attn_n[:, r * B:(r + 1) * B],
                             wo_all[:, r * H + n * 512:r * H + (n + 1) * 512],
                             start=(r == 0), stop=(r == R - 1))
        po = wop.tile([B, 512], F32, tag="po")
        nc.vector.tensor_copy(po[:, :], ps_o[:, :])
        nc.sync.dma_start(partial_d[:, n * 512:(n + 1) * 512], po[:, :])
    nc.gpsimd.collective_compute(
        "ReduceScatter", mybir.AluOpType.add,
        replica_groups=[list(range(NCORE))],
        ins=[partial_d.opt()], outs=[rs_out.opt()])
    nc.sync.dma_start(out_ext[:, :], rs_out[:, :])


def build_nc(num_devices: int = NCORE):
    nc = bacc.Bacc("TRN2", target_bir_lowering=False, debug=False,
                   num_devices=num_devices)
    nch = T0 // 128
    io = {
        # xT pre-tiled: [128, nhch*B], col block h = x h-chunk [128, B]
        "xT": nc.dram_tensor("xT", [128, (H // 128) * B], BF16,
                             kind="ExternalInput").ap(),
        # wqkv pre-tiled: [128, nhch*HL], col block h = w chunk [128, HL]
        "wqkv": nc.dram_tensor("wqkv", [128, (H // 128) * HL], BF16,
                               kind="ExternalInput").ap(),
        "wo": nc.dram_tensor("wo", [R * HD, H], BF16, kind="ExternalInput").ap(),
        # K cache transposed + group-packed: [B//KG, HD, KG*T0],
        # [bg, d, j*T0:(j+1)*T0] = K[bg*KG+j, :, d-th dim... (see shard_inputs)
        "k8T": nc.dram_tensor("k8T", [B // 8, HD, 8 * T0], I8,
                              kind="ExternalInput").ap(),
        "skT": nc.dram_tensor("skT", [B, HD // G, T0], BF16,
                              kind="ExternalInput").ap(),
        # v8 pre-tiled: [B, 128, nch*HD]: [b, p, tc*128:+128] = v8[b, tc*128+p, :]
        "v8": nc.dram_tensor("v8", [B, 128, nch * HD], I8,
                             kind="ExternalInput").ap(),
        "sv": nc.dram_tensor("sv", [B, 128, nch * (HD // G)], BF16,
                             kind="ExternalInput").ap(),
        "cs": nc.dram_tensor("cs", [2, 64], F32, kind="ExternalInput").ap(),
        "eexp": nc.dram_tensor("eexp", [16, 128], BF16,
                               kind="ExternalInput").ap(),
        "out": nc.dram_tensor("out", [B // NCORE, H], F32,
                              kind="ExternalOutput").ap(),
    }
    with tile.TileContext(nc) as tc:
        with ExitStack() as ctx:
            _emit(ctx, tc, io)
    nc.compile()
    return nc


def shard_inputs(x, wqkv, wo, kv_cache, kv_scale, start_pos):
    """Host-side sharding + layout prep. Returns list of per-core input dicts."""
    pos = float(int(start_pos))
    half = HD // 2
    inv_freq = 1.0 / (THETA ** (np.arange(half, dtype=np.float64) / half))
    ang = pos * inv_freq
    cs = np.stack([np.cos(ang), np.sin(ang)]).astype(np.float32)
    eexp = np.zeros((16, 128), dtype=bf16)
    for g in range(16):
        eexp[g, g * G:(g + 1) * G] = 1.0

    nch = T0 // 128
    nhch = H // 128
    # x transposed + tiled: [128, nhch*B]
    xT = np.ascontiguousarray(
        x[:, 0, :].T.reshape(nhch, 128, B).transpose(1, 0, 2).reshape(
            128, nhch * B)).astype(bf16)
    in_maps = []
    for c in range(NCORE):
        qcols = wqkv[:, c * R * HD:(c + 1) * R * HD]
        kcols = wqkv[:, NH * HD + c * HD: NH * HD + (c + 1) * HD]
        vcols = wqkv[:, (NH + NKV) * HD + c * HD: (NH + NKV) * HD + (c + 1) * HD]
        wqkv_l = np.concatenate([qcols, kcols, vcols], axis=1)        # [H, HL]
        wqkv_t = np.ascontiguousarray(
            wqkv_l.reshape(nhch, 128, HL).transpose(1, 0, 2).reshape(
                128, nhch * HL)).astype(bf16)
        wo_l = np.ascontiguousarray(wo[c * R * HD:(c + 1) * R * HD, :]).astype(bf16)
        KG = 8
        k8T = np.ascontiguousarray(
            kv_cache[0, :, c].transpose(0, 2, 1)                      # [B,HD,T0]
            .reshape(B // KG, KG, HD, T0).transpose(0, 2, 1, 3)
            .reshape(B // KG, HD, KG * T0))
        skT = np.ascontiguousarray(
            kv_scale[0, :, c].transpose(0, 2, 1)).astype(bf16)            # [B,16,T0]
        # v8/sv pre-tiled: [B, 128, nch*{HD,16}]
        v8 = np.ascontiguousarray(
            kv_cache[1, :, c].reshape(B, nch, 128, HD).transpose(0, 2, 1, 3)
            .reshape(B, 128, nch * HD))
        sv = np.ascontiguousarray(
            kv_scale[1, :, c].reshape(B, nch, 128, HD // G)
            .transpose(0, 2, 1, 3).reshape(B, 128, nch * (HD // G))).astype(bf16)
        in_maps.append({
            "xT": xT, "wqkv": wqkv_t, "wo": wo_l,
            "k8T": k8T, "skT": skT, "v8": v8, "sv": sv, "cs": cs, "eexp": eexp,
        })
    return in_maps


_NC_CACHE = {}


def kernel(x, wqkv, wo, kv_cache, kv_scale, start_pos):
    in_maps = shard_inputs(x, wqkv, wo, kv_cache, kv_scale, start_pos)
    if "nc" not in _NC_CACHE:
        _NC_CACHE["nc"] = build_nc()
    nc = _NC_CACHE["nc"]
    res = run_bass_kernel_spmd(nc, in_maps, list(range(NCORE)))
    outs = [res.results[i]["out"] for i in range(NCORE)]
    full = np.concatenate(outs, axis=0).astype(np.float32)        # [B, H]
    return full.reshape(B, 1, H)


# revision 39
# speedup vs baseline: 1.0711x; 1.0711x over previous
"""Self-contained Trainium2 Bass kernel for GQA int8-KV-cache decode attention.

Full inputs -> shard over 8 cores (1 kv head + 4 q heads per core) ->
Bass/Tile kernel (QKV proj, RoPE, dequant, attention, out proj) ->
ReduceScatter over cores -> host concat.
"""
import math
from contextlib import ExitStack

import numpy as np
import ml_dtypes

import concourse.bass as bass
import concourse.tile as tile
from concourse import bacc, mybir, masks
from concourse.bass_utils import run_bass_kernel_spmd

bf16 = ml_dtypes.bfloat16
F32, BF16, I8 = mybir.dt.float32, mybir.dt.bfloat16, mybir.dt.int8

# Problem dims (hardcoded per spec)
B, H, NH, NKV, HD, G, T0 = 32, 4096, 32, 8, 128, 8, 4096
THETA = 10000.0
NCORE = 8
R = NH // NCORE            # q heads per core = 4
HL = (R + 2) * HD          # local qkv out cols = 768
NCH = T0 // 128            # past-token chunks = 32
PCOL = (NCH + 1) * R       # score cols = 132 (32 past chunks + 1 new) * 4
SUPER = 1024               # t superchunk size
NSUP = T0 // SUPER         # 4
INV_SQRT_HD = 1.0 / math.sqrt(HD)
# Of every 8 batches, route this many V-dequants to GPSIMD (rest on DVE)
GPSIMD_V_NB8 = 8


def set_dims(t0, super_):
    """Override token dims (for scaled-down simulation tests)."""
    global T0, SUPER, NCH, PCOL, NSUP
    T0, SUPER = t0, super_
    NCH = T0 // 128
    PCOL = (NCH + 1) * R
    NSUP = T0 // SUPER


def _emit(ctx: ExitStack, tc: tile.TileContext, io: dict):
    nc = tc.nc
    xT, wqkv, wo = io["xT"], io["wqkv"], io["wo"]
    k8T, skT, v8, sv, cs = io["k8T"], io["skT"], io["v8"], io["sv"], io["cs"]
    out_ext = io["out"]

    nsup = T0 // SUPER
    nch_sup = SUPER // 128          # chunks per superchunk = 8

    # ---------------- pools
    cpool = ctx.enter_context(tc.tile_pool(name="const", bufs=1))
    apool = ctx.enter_context(tc.tile_pool(name="phaseA", bufs=1))
    xw = ctx.enter_context(tc.tile_pool(name="xw", bufs=2))
    kp = ctx.enter_context(tc.tile_pool(name="kp", bufs=2))
    kgp = ctx.enter_context(tc.tile_pool(name="kgp", bufs=2))
    vp = ctx.enter_context(tc.tile_pool(name="vp", bufs=2))
    pp = ctx.enter_context(tc.tile_pool(name="pp", bufs=3))
    wop = ctx.enter_context(tc.tile_pool(name="wop", bufs=2))
    dram = ctx.enter_context(tc.tile_pool(name="dram", bufs=1, space="DRAM"))

    ps_io = ctx.enter_context(tc.tile_pool(name="ps_io", bufs=1, space="PSUM"))
    ps_skf = ctx.enter_context(tc.tile_pool(name="ps_skf", bufs=2, space="PSUM"))
    ps_sc = ctx.enter_context(tc.tile_pool(name="ps_sc", bufs=2, space="PSUM"))
    ps_at = ctx.enter_context(tc.tile_pool(name="ps_at", bufs=2, space="PSUM"))

    # ---------------- constants
    iden = cpool.tile([128, 128], F32)
    masks.make_identity(nc, iden[:, :])
    ones = cpool.tile([128, 1], BF16)
    nc.vector.memset(ones[:, :], 1.0)
    cosb = cpool.tile([B, 64], F32)
    sinb = cpool.tile([B, 64], F32)
    nc.sync.dma_start(cosb[:, :], cs[0:1, :].unsqueeze(1).broadcast_to([1, B, 64]))
    nc.sync.dma_start(sinb[:, :], cs[1:2, :].unsqueeze(1).broadcast_to([1, B, 64]))

    eexp = cpool.tile([16, 128], BF16)         # E[g,d]=1 iff d//8==g
    nc.sync.dma_start(eexp[:, :], io["eexp"][:, :])
    qT = cpool.tile([128, B * R], BF16)        # cols b*4+r
    kTn = cpool.tile([128, B], BF16)           # new-token K^T
    vnew = cpool.tile([B, 128], BF16)          # new-token V rows
    attn_n = cpool.tile([128, B * R], BF16)    # normalized attn, cols r*32+b
    wo_all = cpool.tile([128, R * H], BF16)    # preloaded wo rows
    vd_last = cpool.tile([128, 128], BF16)     # per-b last V chunk (row 0 only)
    nc.vector.memset(vd_last[:, :], 0.0)

    # ---------------- prefetch first K-scale tiles before the weight stream
    skc_pre = []
    for b0 in range(2):
        skc = kp.tile([16, T0], BF16, tag="sk")
        nc.scalar.dma_start(skc[:, :], skT[b0, :, :])
        skc_pre.append(skc)

    # ---------------- phase A: QKV projection
    ps_qkv = ps_io.tile([B, HL], F32, tag="io")
    nhch = H // 128
    xc_all = apool.tile([128, nhch * B], BF16)   # col block h: x chunk h
    nc.scalar.dma_start(xc_all[:, :], xT[:, :])
    WGRP = 8                                     # h-chunks per w DMA
    for hg in range(nhch // WGRP):
        wc = xw.tile([128, WGRP * HL], BF16, tag="w")
        nc.scalar.dma_start(wc[:, :],
                             wqkv[:, hg * WGRP * HL:(hg + 1) * WGRP * HL])
        for hh in range(WGRP):
            h = hg * WGRP + hh
            xcv = xc_all[:, h * B:(h + 1) * B]
            wcv = wc[:, hh * HL:(hh + 1) * HL]
            nc.tensor.matmul(ps_qkv[:, 0:512], xcv, wcv[:, 0:512],
                             start=(h == 0), stop=(h == nhch - 1))
            nc.tensor.matmul(ps_qkv[:, 512:768], xcv, wcv[:, 512:768],
                             start=(h == 0), stop=(h == nhch - 1))

    qkv_sb = apool.tile([B, HL], F32)
    nc.vector.tensor_copy(qkv_sb[:, :], ps_qkv[:, :])

    # ---------------- phase A: RoPE on q (4 heads) + k (1 head)
    rope = apool.tile([B, 5 * 128], F32)
    t1 = qkv_sb[:, 0:640].rearrange("b (h c) -> b h c", h=5)[:, :, 0:64]
    t2 = qkv_sb[:, 0:640].rearrange("b (h c) -> b h c", h=5)[:, :, 64:128]
    o1 = rope[:, :].rearrange("b (h c) -> b h c", h=5)[:, :, 0:64]
    o2 = rope[:, :].rearrange("b (h c) -> b h c", h=5)[:, :, 64:128]
    cos3 = cosb[:, :].unsqueeze(1).broadcast_to([B, 5, 64])
    sin3 = sinb[:, :].unsqueeze(1).broadcast_to([B, 5, 64])
    m1 = apool.tile([B, 5 * 64], F32)
    m2 = apool.tile([B, 5 * 64], F32)
    m1v = m1[:, :].rearrange("b (h c) -> b h c", h=5)
    m2v = m2[:, :].rearrange("b (h c) -> b h c", h=5)
    nc.vector.tensor_mul(m1v, t1, cos3)
    nc.vector.tensor_mul(m2v, t2, sin3)
    nc.vector.tensor_sub(o1, m1v, m2v)
    nc.vector.tensor_mul(m1v, t2, cos3)
    nc.vector.tensor_mul(m2v, t1, sin3)
    nc.vector.tensor_add(o2, m1v, m2v)

    # ---------------- phase A: transposes (q heads + new k), v_new cast
    for r in range(R):
        ps_t = ps_io.tile([128, B], F32, tag="io")
        nc.tensor.transpose(ps_t[:, :], rope[:, r * 128:(r + 1) * 128],
                            iden[0:B, 0:B])
        qT_view = qT[:, :].rearrange("d (b r) -> d b r", r=R)[:, :, r]
        nc.vector.tensor_copy(qT_view, ps_t[:, :])
    ps_t = ps_io.tile([128, B], F32, tag="io")
    nc.tensor.transpose(ps_t[:, :], rope[:, 512:640], iden[0:B, 0:B])
    nc.vector.tensor_copy(kTn[:, :], ps_t[:, :])
    nc.vector.tensor_copy(vnew[:, :], qkv_sb[:, 640:768])

    # ---------------- phase B: per-batch attention
    KG = 8
    k8g = None
    for b in range(B):
        # --- K path: dequant + scores
        if b % KG == 0:
            k8g = kgp.tile([128, KG * T0], I8, tag="k8")
            nc.sync.dma_start(k8g[:, :], k8T[b // KG, :, :])
        if b == 2:
            for r in range(R):
                nc.sync.dma_start(wo_all[:, r * H:(r + 1) * H],
                                  wo[r * 128:(r + 1) * 128, :])
        ps_s = ps_sc.tile([128, 2 * PCOL], F32, tag="sc")
        k8c = k8g[:, (b % KG) * T0:(b % KG + 1) * T0]
        if b < 2:
            skc = skc_pre[b]
        else:
            skc = kp.tile([16, T0], BF16, tag="sk")
            nc.scalar.dma_start(skc[:, :], skT[b, :, :])
        kd = kp.tile([128, T0], BF16, tag="kd")
        for chk in range(T0 // 512):
            skf_ps = ps_skf.tile([128, 512], F32, tag="skf")
            nc.tensor.matmul(skf_ps[:, :], eexp[:, :],
                             skc[:, chk * 512:(chk + 1) * 512],
                             start=True, stop=True)
            nc.vector.tensor_mul(kd[:, chk * 512:(chk + 1) * 512],
                                 k8c[:, chk * 512:(chk + 1) * 512],
                                 skf_ps[:, :])
        for ch in range(NCH):
            nc.tensor.matmul(ps_s[:, ch * R:(ch + 1) * R],
                             kd[:, ch * 128:(ch + 1) * 128],
                             qT[:, b * R:(b + 1) * R],
                             start=True, stop=True)
        # new-token score: row 0 of last col-block; rest = -1e30 -> exp 0
        nc.vector.memset(ps_s[:, NCH * R:PCOL], -1e30)
        nc.tensor.matmul(ps_s[0:1, NCH * R:PCOL], kTn[:, b:b + 1],
                         qT[:, b * R:(b + 1) * R], start=True, stop=True)

        # --- softmax (unnormalized): p = exp(scores/sqrt(HD))
        p_b = pp.tile([128, PCOL], BF16, tag="p")
        nc.scalar.activation(p_b[:, :], ps_s[:, 0:PCOL],
                             mybir.ActivationFunctionType.Exp,
                             scale=INV_SQRT_HD)
        # column sums via ones-matmul, then fold chunks, reciprocal
        ps_m = ps_s[0:1, PCOL:2 * PCOL]
        nc.tensor.matmul(ps_m, ones[:, :], p_b[:, :], start=True, stop=True)
        red = pp.tile([1, R], F32, tag="red")
        nc.vector.tensor_reduce(red[0:1, :],
                                ps_m.rearrange("p (c r) -> p r c", r=R),
                                axis=mybir.AxisListType.X, op=mybir.AluOpType.add)
        rec4 = pp.tile([1, R], F32, tag="rec")
        nc.vector.reciprocal(rec4[0:1, :], red[0:1, :])
        rec4b = pp.tile([128, R], F32, tag="recb")
        nc.sync.dma_start(rec4b[:, :],
                          rec4[0:1, :].unsqueeze(1).broadcast_to([1, 128, R]))

        # --- V path: dequant + attention matmul (DMAs issued from ACT queue)
        ps_a = ps_at.tile([128, R], F32, tag="at")
        v8c = vp.tile([128, T0], I8, tag="v8")
        nc.scalar.dma_start(v8c[:, :], v8[b, :, :])
        svc = vp.tile([128, NCH * 16], BF16, tag="sv")
        nc.scalar.dma_start(svc[:, :], sv[b, :, :])
        vd = vp.tile([128, T0], BF16, tag="vd")
        eng = nc.gpsimd if (b % 8) < GPSIMD_V_NB8 else nc.vector
        eng.tensor_mul(
            vd[:, :].rearrange("p (s e) -> p s e", e=G),
            v8c[:, :].rearrange("p (s e) -> p s e", e=G),
            svc[:, :].unsqueeze(2).broadcast_to([128, NCH * 16, G]))
        for ch in range(NCH):
            nc.tensor.matmul(ps_a[:, :], vd[:, ch * 128:(ch + 1) * 128],
                             p_b[:, ch * R:(ch + 1) * R],
                             start=(ch == 0), stop=False)
        # new-token V contribution
        nc.scalar.dma_start(vd_last[0:1, :], vnew[b:b + 1, :])
        nc.tensor.matmul(ps_a[:, :], vd_last[:, :], p_b[:, NCH * R:PCOL],
                         start=False, stop=True)
        at_view = attn_n[:, :].rearrange("d (r b) -> d r b", b=B)[:, :, b]
        nc.vector.tensor_mul(at_view, ps_a[:, :], rec4b[:, :])

    # ---------------- phase C: output projection + collective
    partial_d = dram.tile([B, H], F32)
    rs_out = dram.tile([B // NCORE, H], F32)
    for n in range(H // 512):
        ps_o = ps_io.tile([B, 512], F32, tag="io")
        for r in range(R):
            nc.tensor.matmul(ps_o[:, :], attn_n[:, r * B:(r + 1) * B],
                             wo_all[:, r * H + n * 512:r * H + (n + 1) * 512],
                             start=(r == 0), stop=(r == R - 1))
        po = wop.tile([B, 512], F32, tag="po")
        nc.vector.tensor_copy(po[:, :], ps_o[:, :])
        nc.sync.dma_start(partial_d[:, n * 512:(n + 1) * 512], po[:, :])
    nc.gpsimd.collective_compute(
        "ReduceScatter", mybir.AluOpType.add,
        replica_groups=[list(range(NCORE))],
        ins=[partial_d.opt()], outs=[rs_out.opt()])
    nc.sync.dma_start(out_ext[:, :], rs_out[:, :])


def build_nc(num_devices: int = NCORE):
    nc = bacc.Bacc("TRN2", target_bir_lowering=False, debug=False,
                   num_devices=num_devices)
    nch = T0 // 128
    io = {
        # xT pre-tiled: [128, nhch*B], col block h = x h-chunk [128, B]
        "xT": nc.dram_tensor("xT", [128, (H // 128) * B], BF16,
                             kind="ExternalInput").ap(),
        # wqkv pre-tiled: [128, nhch*HL], col block h = w chunk [128, HL]
        "wqkv": nc.dram_tensor("wqkv", [128, (H // 128) * HL], BF16,
                               kind="ExternalInput").ap(),
        "wo": nc.dram_tensor("wo", [R * HD, H], BF16, kind="ExternalInput").ap(),
        # K cache transposed + group-packed: [B//KG, HD, KG*T0],
        # [bg, d, j*T0:(j+1)*T0] = K[bg*KG+j, :, d-th dim... (see shard_inputs)
        "k8T": nc.dram_tensor("k8T", [B // 8, HD, 8 * T0], I8,
                              kind="ExternalInput").ap(),
        "skT": nc.dram_tensor("skT", [B, HD // G, T0], BF16,
                              kind="ExternalInput").ap(),
        # v8 pre-tiled: [B, 128, nch*HD]: [b, p, tc*128:+128] = v8[b, tc*128+p, :]
        "v8": nc.dram_tensor("v8", [B, 128, nch * HD], I8,
                             kind="ExternalInput").ap(),
        "sv": nc.dram_tensor("sv", [B, 128, nch * (HD // G)], BF16,
                             kind="ExternalInput").ap(),
        "cs": nc.dram_tensor("cs", [2, 64], F32, kind="ExternalInput").ap(),
        "eexp": nc.dram_tensor("eexp", [16, 128], BF16,
                               kind="ExternalInput").ap(),
        "out": nc.dram_tensor("out", [B // NCORE, H], F32,
                              kind="ExternalOutput").ap(),
    }
    with tile.TileContext(nc) as tc:
        with ExitStack() as ctx:
            _emit(ctx, tc, io)
    nc.compile()
    return nc


def shard_inputs(x, wqkv, wo, kv_cache, kv_scale, start_pos):
    """Host-side sharding + layout prep. Returns list of per-core input dicts."""
    pos = float(int(start_pos))
    half = HD // 2
    inv_freq = 1.0 / (THETA ** (np.arange(half, dtype=np.float64) / half))
    ang = pos * inv_freq
    cs = np.stack([np.cos(ang), np.sin(ang)]).astype(np.float32)
    eexp = np.zeros((16, 128), dtype=bf16)
    for g in range(16):
        eexp[g, g * G:(g + 1) * G] = 1.0

    nch = T0 // 128
    nhch = H // 128
    # x transposed + tiled: [128, nhch*B]
    xT = np.ascontiguousarray(
        x[:, 0, :].T.reshape(nhch, 128, B).transpose(1, 0, 2).reshape(
            128, nhch * B)).astype(bf16)
    in_maps = []
    for c in range(NCORE):
        qcols = wqkv[:, c * R * HD:(c + 1) * R * HD]
        kcols = wqkv[:, NH * HD + c * HD: NH * HD + (c + 1) * HD]
        vcols = wqkv[:, (NH + NKV) * HD + c * HD: (NH + NKV) * HD + (c + 1) * HD]
        wqkv_l = np.concatenate([qcols, kcols, vcols], axis=1)        # [H, HL]
        wqkv_t = np.ascontiguousarray(
            wqkv_l.reshape(nhch, 128, HL).transpose(1, 0, 2).reshape(
                128, nhch * HL)).astype(bf16)
        wo_l = np.ascontiguousarray(wo[c * R * HD:(c + 1) * R * HD, :]).astype(bf16)
        KG = 8
        k8T = np.ascontiguousarray(
            kv_cache[0, :, c].transpose(0, 2, 1)                      # [B,HD,T0]
            .reshape(B // KG, KG, HD, T0).transpose(0, 2, 1, 3)
            .reshape(B // KG, HD, KG * T0))
        skT = np.ascontiguousarray(
            kv_scale[0, :, c].transpose(0, 2, 1)).astype(bf16)            # [B,16,T0]
        # v8/sv pre-tiled: [B, 128, nch*{HD,16}]
        v8 = np.ascontiguousarray(
            kv_cache[1, :, c].reshape(B, nch, 128, HD).transpose(0, 2, 1, 3)
            .reshape(B, 128, nch * HD))
        sv = np.ascontiguousarray(
            kv_scale[1, :, c].reshape(B, nch, 128, HD // G)
            .transpose(0, 2, 1, 3).reshape(B, 128, nch * (HD // G))).astype(bf16)
        in_maps.append({
            "xT": xT, "wqkv": wqkv_t, "wo": wo_l,
            "k8T": k8T, "skT": skT, "v8": v8, "sv": sv, "cs": cs, "eexp": eexp,
        })
    return in_maps


_NC_CACHE = {}


def kernel(x, wqkv, wo, kv_cache, kv_scale, start_pos):
    in_maps = shard_inputs(x, wqkv, wo, kv_cache, kv_scale, start_pos)
    if "nc" not in _NC_CACHE:
        _NC_CACHE["nc"] = build_nc()
    nc = _NC_CACHE["nc"]
    res = run_bass_kernel_spmd(nc, in_maps, list(range(NCORE)))
    outs = [res.results[i]["out"] for i in range(NCORE)]
    full = np.concatenate(outs, axis=0).astype(np.float32)        # [B, H]
    return full.reshape(B, 1, H)
